# revision 23
# baseline (speedup 1.0000x reference)
"""Trainium2 Bass kernel for GNN message passing (8 NeuronCores, SPMD).

    out = segment_sum(x[src] @ W, tgt, N) + x @ W_self

Key algebraic identity: segment_sum(x[src] @ W, tgt) = segment_sum(x[src], tgt) @ W,
so the per-edge matmul hoists out of the reduction (21 GFLOP -> 6.6 GFLOP).

Sharding: target nodes are split into 8 contiguous ranges of 12500 (one per
core); edges are bucketed to the core owning their target. x is replicated in
every core's HBM so each core gathers arbitrary source rows locally (no
cross-core halo exchange needed under the full-I/O contract).

Per core, working transposed throughout (out.T = W.T @ hT + W_self.T @ xT):
  - targets are processed in 98 windows of 128 nodes
  - per 128-edge tile: G[e,f] = x[src_e] via indirect DMA gather,
    S[e,n] = onehot(tgt_local) built by a DVE is_equal against an iota,
    hT (PSUM) += matmul(lhsT=G, rhs=S)   # = sum_e G[e,f] S[e,n]
  - outT (PSUM) = matmul(lhsT=W, rhs=hT) + matmul(lhsT=W_self, rhs=xT_window)
The host transposes per-core [128, 12544] outputs back and concatenates.
"""

import numpy as np

P = 128
D = 128
N_NODES = 100000
N_CORES = 8
N_LOC = N_NODES // N_CORES          # 12500
N_WIN = (N_LOC + P - 1) // P        # 98
N_PAD = N_WIN * P                   # 12544

# dma_gather uses int16 row indices, so x is addressed through 4 overlapping
# 32768-row chunks; every source row is reachable from >=1 chunk and rows in
# overlap regions can go to either side, which lets the host balance the four
# per-window runs under the per-chunk tile cap.
N_CHUNK = 4
CHUNK_SPAN = 32768
CHUNK_BASE = [0, 22411, 44822, N_NODES - CHUNK_SPAN]

_program_cache: dict = {}


def _build_program(t_win: int, reps: int = 1):
    import concourse.bass as bass
    import concourse.mybir as mybir
    import concourse.tile as tile
    from concourse.bacc import Bacc

    f32 = mybir.dt.float32
    t_tot = N_WIN * t_win

    # consts packed as one tensor/DMA so consumers wait on a single semaphore:
    # [idx16 (int16 bits) | tl | iota | W | W_self] along the free dim
    t_c = t_win // N_CHUNK
    idx_cols16 = N_WIN * N_CHUNK * t_c * 8   # int16 columns
    idx_cols = idx_cols16 // 2               # as float32 columns
    k_const = idx_cols + t_tot + 3 * P

    # Bacc (not raw Bass): its finalize() legalizes sync waits — TRN2 allows
    # at most one semaphore wait per instruction and walrus rejects more.
    nc = Bacc()
    x_d = nc.declare_dram_parameter("x", [N_NODES, D], f32, isOutput=False)
    xT_d = nc.declare_dram_parameter("xT", [D, N_PAD], f32, isOutput=False)
    consts_d = nc.declare_dram_parameter(
        "consts", [P, k_const], mybir.dt.int32, isOutput=False
    )
    outT_d = nc.declare_dram_parameter("outT", [D, N_PAD], f32, isOutput=True)

    with tile.TileContext(nc) as tc:
        with (
            tc.tile_pool(name="const", bufs=1) as cpool,
            tc.tile_pool(name="gath", bufs=16) as gpool,
            tc.tile_pool(name="spool", bufs=3) as spool,
            tc.tile_pool(name="wtile", bufs=3) as wpool,
            tc.tile_pool(name="psum", bufs=2, space="PSUM") as psum,
            tc.tile_pool(name="opsum", bufs=2, space="PSUM") as opsum,
            tc.tile_pool(name="scratch", bufs=1, space="PSUM") as scratch_pool,
        ):
            scratch_ps = scratch_pool.tile([1, 1], f32)
            const_sb = cpool.tile([P, k_const], mybir.dt.int32)
            nc.sync.dma_start(const_sb[:], consts_d[:])
            idx16_sb = const_sb[:, 0:idx_cols].bitcast(mybir.dt.int16)
            tl_sb = const_sb[:, idx_cols : idx_cols + t_tot].bitcast(f32)
            iota_sb = const_sb[:, idx_cols + t_tot : idx_cols + t_tot + P].bitcast(f32)
            w_sb = const_sb[:, idx_cols + t_tot + P : idx_cols + t_tot + 2 * P].bitcast(
                f32
            )
            ws_sb = const_sb[
                :, idx_cols + t_tot + 2 * P : idx_cols + t_tot + 3 * P
            ].bitcast(f32)

            for w in [w for _ in range(reps) for w in range(N_WIN)]:
                hT_ps = psum.tile([D, P], f32)
                S_big = spool.tile([P, t_win, P], f32)
                nc.vector.tensor_tensor(
                    out=S_big[:],
                    in0=tl_sb[:, w * t_win : (w + 1) * t_win, None].to_broadcast(
                        [P, t_win, P]
                    ),
                    in1=iota_sb[:, None, :].to_broadcast([P, t_win, P]),
                    op=mybir.AluOpType.is_equal,
                )
                # fp32 matmuls are single fused instructions that can carry only
                # ONE sync wait; the first real matmul below depends on both
                # S_big (DVE) and G_big (DMA). This throwaway 1x1 matmul makes
                # the PE queue observe the DVE tick first so each real matmul
                # needs a single wait.
                nc.tensor.matmul(
                    scratch_ps[:],
                    lhsT=S_big[:, 0, 0:1],
                    rhs=S_big[:, 0, 0:1],
                    start=True,
                    stop=True,
                )
                # gather via dma_gather (int16 idx against a 32768-row chunk of
                # x): G_big slot (p, t) = row idx[t*128+p] of the chunk. Much
                # cheaper on the GPSIMD Q7 descriptor generator than per-tile
                # indirect_dma_start. (A single batched indirect DMA with a
                # [128, t_win] offset AP computes garbage on real HW.)
                G_big = gpool.tile([P, t_win, D], f32)
                for c in range(N_CHUNK):
                    g = w * N_CHUNK + c
                    nc.gpsimd.dma_gather(
                        G_big[:, c * t_c : (c + 1) * t_c, :],
                        x_d[CHUNK_BASE[c] : CHUNK_BASE[c] + CHUNK_SPAN, :],
                        idx16_sb[:, g * t_c * 8 : (g + 1) * t_c * 8],
                        t_c * P,
                        t_c * P,
                        D,
                    )
                for t in range(t_win):
                    nc.tensor.matmul(
                        hT_ps[:],
                        lhsT=G_big[:, t, :],
                        rhs=S_big[:, t, :],
                        start=(t == 0),
                        stop=(t == t_win - 1),
                    )
                hT_sb = wpool.tile([D, P], f32, tag="hT")
                nc.vector.tensor_copy(hT_sb[:], hT_ps[:])
                xT_sb = wpool.tile([D, P], f32, tag="xT")
                nc.sync.dma_start(xT_sb[:], xT_d[:, w * P : (w + 1) * P])
                outT_ps = opsum.tile([D, P], f32)
                nc.tensor.matmul(
                    outT_ps[:], lhsT=w_sb, rhs=hT_sb[:], start=True, stop=False
                )
                nc.tensor.matmul(
                    outT_ps[:], lhsT=ws_sb, rhs=xT_sb[:], start=False, stop=True
                )
                o_sb = wpool.tile([D, P], f32, tag="o")
                nc.vector.tensor_copy(o_sb[:], outT_ps[:])
                nc.sync.dma_start(outT_d[:, w * P : (w + 1) * P], o_sb[:])

    nc.finalize()
    return nc


def _prep_inputs(x, edge_index, W, W_self):
    """Host-side sharding: bucket+sort edges by target core/window, pad to a
    uniform tile count, build per-core input maps."""
    x = np.ascontiguousarray(np.asarray(x, dtype=np.float32))
    W = np.ascontiguousarray(np.asarray(W, dtype=np.float32))
    W_self = np.ascontiguousarray(np.asarray(W_self, dtype=np.float32))
    ei = np.asarray(edge_index)
    src = ei[0].astype(np.int64)
    tgt = ei[1].astype(np.int64)
    E = src.shape[0]

    order = np.argsort(tgt, kind="stable")
    src_s = src[order].astype(np.int64)
    tgt_s = tgt[order]
    core = tgt_s // N_LOC
    wloc = (tgt_s - core * N_LOC) // P
    gw = (core * N_WIN + wloc).astype(np.int64)
    counts = np.bincount(gw, minlength=N_CORES * N_WIN)
    t_win_data = max(1, int(np.ceil(counts.max() / P)))
    t_c = max(2, (t_win_data + N_CHUNK - 1) // N_CHUNK)

    # chunk feasibility per edge: lo = highest chunk with base <= s,
    # hi = lowest chunk with s < base + CHUNK_SPAN (consecutive range)
    bases = np.asarray(CHUNK_BASE, np.int64)
    lo = np.searchsorted(bases, src_s, side="right") - 1
    hi = np.searchsorted(bases + CHUNK_SPAN, src_s, side="right")
    starts = np.concatenate([[0], np.cumsum(counts)])
    tl_val = (tgt_s - (core * N_LOC + wloc * P)).astype(np.float32)

    while True:
        cap = t_c * P
        t_win = N_CHUNK * t_c
        t_tot = N_WIN * t_win
        idx16 = np.zeros((N_CORES, N_WIN * N_CHUNK * cap // 16, 16), np.int16)
        tl_flat = np.full(N_CORES * t_tot * P, -1.0, np.float32)
        ok = True
        for g in range(N_CORES * N_WIN):
            a, b = starts[g], starts[g + 1]
            if b - a > N_CHUNK * cap:
                ok = False
                break
            s_g, hi_g, lo_g, tl_g = src_s[a:b], hi[a:b], lo[a:b], tl_val[a:b]
            taken = np.zeros(b - a, bool)
            c_core, w = divmod(g, N_WIN)
            for c in range(N_CHUNK):
                cand = (~taken) & (hi_g <= c) & (c <= lo_g)
                must = cand & (lo_g == c)
                n_must = int(must.sum())
                if n_must > cap:
                    ok = False
                    break
                sel = must.nonzero()[0]
                flex = (cand & ~must).nonzero()[0][: cap - n_must]
                pick = np.concatenate([sel, flex])
                taken[pick] = True
                n = pick.size
                idx = (s_g[pick] - bases[c]).astype(np.int16)
                # wrapped int16 layout: slot s -> [s % 16, s // 16]
                blk = np.zeros(cap, np.int16)
                blk[:n] = idx
                row0 = (w * N_CHUNK + c) * (cap // 16)
                idx16[c_core, row0 : row0 + cap // 16] = blk.reshape(cap // 16, 16)
                # tl slots for this chunk run (pads stay -1)
                base_slot = g * (t_win * P) + c * cap
                tl_flat[base_slot : base_slot + n] = tl_g[pick]
            if not ok or not taken.all():
                ok = ok and bool(taken.all())
                if not ok:
                    break
        if ok:
            break
        t_c += 1

    tl_dev = tl_flat.reshape(N_CORES, t_tot, P).transpose(0, 2, 1)
    iota = np.tile(np.arange(P, dtype=np.float32), (P, 1))
    in_maps = []
    for c in range(N_CORES):
        # idx16[c]: [n16, 16] with slot s of block g at [g*cap/16 + s%16 ...]
        # -> SBUF layout [128 partitions, cols]: block g occupies columns
        # [g*t_c*8, (g+1)*t_c*8), partition rows 0..15
        n_blocks = N_WIN * N_CHUNK
        cols16 = t_c * 8
        a = idx16[c].reshape(n_blocks, cap // 16, 16)  # [g, col, row]
        # [16, cols] block replicated across all 8 GPSIMD Q7 cores' stripes
        sb = np.tile(a.transpose(2, 0, 1).reshape(16, n_blocks * cols16), (8, 1))
        xT_c = np.zeros((D, N_PAD), np.float32)
        xT_c[:, :N_LOC] = x[c * N_LOC : (c + 1) * N_LOC].T
        consts = np.concatenate(
            [
                sb.view(np.int32),
                tl_dev[c].view(np.int32),
                iota.view(np.int32),
                W.view(np.int32),
                W_self.view(np.int32),
            ],
            axis=1,
        )
        in_maps.append({"x": x, "xT": xT_c, "consts": consts})
    return in_maps, t_win


def run(x, edge_index, W, W_self, trace=False, **trace_kwargs):
    """Returns (output [100000,128] float32, BassKernelResults)."""
    from concourse import bass_utils

    in_maps, t_win = _prep_inputs(x, edge_index, W, W_self)
    nc = _program_cache.get(t_win)
    if nc is None:
        nc = _build_program(t_win)
        _program_cache[t_win] = nc
    res = bass_utils.run_bass_kernel_spmd(
        nc, in_maps, core_ids=list(range(N_CORES)), trace=trace, **trace_kwargs
    )
    out = np.empty((N_NODES, D), np.float32)
    for c in range(N_CORES):
        out[c * N_LOC : (c + 1) * N_LOC] = res.results[c]["outT"].T[:N_LOC]
    return out, res


def kernel(x, edge_index, W, W_self):
    out, _ = run(x, edge_index, W, W_self, trace=False)
    return out


# revision 29
# speedup vs baseline: 2.4900x; 2.4900x over previous
"""Trainium2 Bass kernel for GNN message passing (8 NeuronCores, SPMD).

    out = segment_sum(x[src] @ W, tgt, N) + x @ W_self

Key algebraic identity: segment_sum(x[src] @ W, tgt) = segment_sum(x[src], tgt) @ W,
so the per-edge matmul hoists out of the reduction (21 GFLOP -> 6.6 GFLOP).

Sharding: target nodes are split into 8 contiguous ranges of 12500 (one per
core); edges are bucketed to the core owning their target. x is replicated in
every core's HBM so each core gathers arbitrary source rows locally (no
cross-core halo exchange needed under the full-I/O contract).

Per core, working transposed throughout (out.T = W.T @ hT + W_self.T @ xT):
  - targets are processed in 98 windows of 128 nodes
  - per 128-edge tile: G[e,f] = x[src_e] via indirect DMA gather,
    S[e,n] = onehot(tgt_local) built by a DVE is_equal against an iota,
    hT (PSUM) += matmul(lhsT=G, rhs=S)   # = sum_e G[e,f] S[e,n]
  - outT (PSUM) = matmul(lhsT=W, rhs=hT) + matmul(lhsT=W_self, rhs=xT_window)
The host transposes per-core [128, 12544] outputs back and concatenates.
"""

import numpy as np

P = 128
D = 128
N_NODES = 100000
N_CORES = 8
N_LOC = N_NODES // N_CORES          # 12500
N_WIN = (N_LOC + P - 1) // P        # 98
N_PAD = N_WIN * P                   # 12544

# dma_gather uses int16 row indices, so x is addressed through 4 overlapping
# 32768-row chunks; every source row is reachable from >=1 chunk and rows in
# overlap regions can go to either side, which lets the host balance the four
# per-window runs under the per-chunk tile cap.
N_CHUNK = 4
CHUNK_SPAN = 32768
CHUNK_BASE = [0, 22411, 44822, N_NODES - CHUNK_SPAN]

_program_cache: dict = {}


def _build_program(
    t_win: int,
    reps: int = 1,
    n_queues: int = 2,
    act_copy: bool = True,
    w_group: int = 1,
):
    import concourse.bass as bass
    import concourse.mybir as mybir
    import concourse.tile as tile
    from concourse.bacc import Bacc

    f32 = mybir.dt.float32
    t_tot = N_WIN * t_win

    # consts packed as one tensor/DMA so consumers wait on a single semaphore:
    # [idx16 (int16 bits) | tl | iota | W | W_self] along the free dim
    t_c = t_win // N_CHUNK
    idx_cols16 = N_WIN * N_CHUNK * t_c * 8   # int16 columns
    idx_cols = idx_cols16 // 2               # as float32 columns
    k_const = idx_cols + t_tot + 3 * P

    # Bacc (not raw Bass): its finalize() legalizes sync waits — TRN2 allows
    # at most one semaphore wait per instruction and walrus rejects more.
    nc = Bacc(num_swdge_queues=n_queues)
    x_d = nc.declare_dram_parameter("x", [N_NODES, D], f32, isOutput=False)
    xT_d = nc.declare_dram_parameter("xT", [D, N_PAD], f32, isOutput=False)
    consts_d = nc.declare_dram_parameter(
        "consts", [P, k_const], mybir.dt.int32, isOutput=False
    )
    outT_d = nc.declare_dram_parameter("outT", [D, N_PAD], f32, isOutput=True)

    with tile.TileContext(nc) as tc:
        with (
            tc.tile_pool(name="const", bufs=1) as cpool,
            tc.tile_pool(name="gath", bufs=16) as gpool,
            tc.tile_pool(name="spool", bufs=3) as spool,
            tc.tile_pool(name="wtile", bufs=3) as wpool,
            tc.tile_pool(name="psum", bufs=2, space="PSUM") as psum,
            tc.tile_pool(name="opsum", bufs=2, space="PSUM") as opsum,
            tc.tile_pool(name="scratch", bufs=1, space="PSUM") as scratch_pool,
        ):
            scratch_ps = scratch_pool.tile([1, 1], f32)
            const_sb = cpool.tile([P, k_const], mybir.dt.int32)
            nc.sync.dma_start(const_sb[:], consts_d[:])
            idx16_sb = const_sb[:, 0:idx_cols].bitcast(mybir.dt.int16)
            tl_sb = const_sb[:, idx_cols : idx_cols + t_tot].bitcast(f32)
            iota_sb = const_sb[:, idx_cols + t_tot : idx_cols + t_tot + P].bitcast(f32)
            w_sb = const_sb[:, idx_cols + t_tot + P : idx_cols + t_tot + 2 * P].bitcast(
                f32
            )
            ws_sb = const_sb[
                :, idx_cols + t_tot + 2 * P : idx_cols + t_tot + 3 * P
            ].bitcast(f32)

            for w in [w for _ in range(reps) for w in range(N_WIN)]:
                hT_ps = psum.tile([D, P], f32)
                S_big = spool.tile([P, t_win, P], f32)
                nc.vector.tensor_tensor(
                    out=S_big[:],
                    in0=tl_sb[:, w * t_win : (w + 1) * t_win, None].to_broadcast(
                        [P, t_win, P]
                    ),
                    in1=iota_sb[:, None, :].to_broadcast([P, t_win, P]),
                    op=mybir.AluOpType.is_equal,
                )
                # fp32 matmuls are single fused instructions that can carry only
                # ONE sync wait; the first real matmul below depends on both
                # S_big (DVE) and G_big (DMA). This throwaway 1x1 matmul makes
                # the PE queue observe the DVE tick first so each real matmul
                # needs a single wait.
                nc.tensor.matmul(
                    scratch_ps[:],
                    lhsT=S_big[:, 0, 0:1],
                    rhs=S_big[:, 0, 0:1],
                    start=True,
                    stop=True,
                )
                # gather via dma_gather (int16 idx against a 32768-row chunk of
                # x): G_big slot (p, t) = row idx[t*128+p] of the chunk. Much
                # cheaper on the GPSIMD Q7 descriptor generator than per-tile
                # indirect_dma_start. (A single batched indirect DMA with a
                # [128, t_win] offset AP computes garbage on real HW.)
                G_big = gpool.tile([P, t_win, D], f32)
                for c in range(N_CHUNK):
                    g = w * N_CHUNK + c
                    nc.gpsimd.dma_gather(
                        G_big[:, c * t_c : (c + 1) * t_c, :],
                        x_d[CHUNK_BASE[c] : CHUNK_BASE[c] + CHUNK_SPAN, :],
                        idx16_sb[:, g * t_c * 8 : (g + 1) * t_c * 8],
                        t_c * P,
                        t_c * P,
                        D,
                        queue_num=c % n_queues,
                    )
                for t in range(t_win):
                    nc.tensor.matmul(
                        hT_ps[:],
                        lhsT=G_big[:, t, :],
                        rhs=S_big[:, t, :],
                        start=(t == 0),
                        stop=(t == t_win - 1),
                    )
                if w_group == 1:
                    hT_sb = wpool.tile([D, P], f32, tag="hT")
                    nc.vector.tensor_copy(hT_sb[:], hT_ps[:])
                    xT_sb = wpool.tile([D, P], f32, tag="xT")
                    nc.sync.dma_start(xT_sb[:], xT_d[:, w * P : (w + 1) * P])
                    outT_ps = opsum.tile([D, P], f32)
                    nc.tensor.matmul(
                        outT_ps[:], lhsT=w_sb, rhs=hT_sb[:], start=True, stop=False
                    )
                    nc.tensor.matmul(
                        outT_ps[:], lhsT=ws_sb, rhs=xT_sb[:], start=False, stop=True
                    )
                    o_sb = wpool.tile([D, P], f32, tag="o")
                    if act_copy:
                        # ACT is otherwise idle; taking the outT copy off DVE
                        # (which builds every S one-hot) relieves the
                        # 2nd-busiest engine despite slower per-op copies.
                        nc.scalar.copy(o_sb[:], outT_ps[:])
                    else:
                        nc.vector.tensor_copy(o_sb[:], outT_ps[:])
                    nc.sync.dma_start(outT_d[:, w * P : (w + 1) * P], o_sb[:])
                    continue
                # grouped W-apply: stage hT of w_group windows side by side,
                # then stream both weight matmuls at N = w_group*128 to
                # amortize the fp32 weight loads (no FWL for fp32)
                gi = w % w_group
                if gi == 0:
                    n_in_grp = min(w_group, N_WIN - w)
                    hT_sb = wpool.tile([D, w_group * P], f32, tag="hT")
                nc.vector.tensor_copy(
                    hT_sb[:, gi * P : (gi + 1) * P], hT_ps[:]
                )
                if gi == n_in_grp - 1:
                    w0 = w - gi
                    span = n_in_grp * P
                    xT_sb = wpool.tile([D, w_group * P], f32, tag="xT")
                    nc.sync.dma_start(
                        xT_sb[:, :span], xT_d[:, w0 * P : w0 * P + span]
                    )
                    outT_ps = opsum.tile([D, w_group * P], f32)
                    nc.tensor.matmul(
                        outT_ps[:, :span],
                        lhsT=w_sb,
                        rhs=hT_sb[:, :span],
                        start=True,
                        stop=False,
                    )
                    nc.tensor.matmul(
                        outT_ps[:, :span],
                        lhsT=ws_sb,
                        rhs=xT_sb[:, :span],
                        start=False,
                        stop=True,
                    )
                    o_sb = wpool.tile([D, w_group * P], f32, tag="o")
                    if act_copy:
                        nc.scalar.copy(o_sb[:, :span], outT_ps[:, :span])
                    else:
                        nc.vector.tensor_copy(o_sb[:, :span], outT_ps[:, :span])
                    nc.sync.dma_start(
                        outT_d[:, w0 * P : w0 * P + span], o_sb[:, :span]
                    )

    nc.finalize()
    return nc


def _prep_inputs(x, edge_index, W, W_self):
    """Host-side sharding: bucket+sort edges by target core/window, pad to a
    uniform tile count, build per-core input maps."""
    x = np.ascontiguousarray(np.asarray(x, dtype=np.float32))
    W = np.ascontiguousarray(np.asarray(W, dtype=np.float32))
    W_self = np.ascontiguousarray(np.asarray(W_self, dtype=np.float32))
    ei = np.asarray(edge_index)
    src = ei[0].astype(np.int64)
    tgt = ei[1].astype(np.int64)
    E = src.shape[0]

    order = np.argsort(tgt, kind="stable")
    src_s = src[order].astype(np.int64)
    tgt_s = tgt[order]
    core = tgt_s // N_LOC
    wloc = (tgt_s - core * N_LOC) // P
    gw = (core * N_WIN + wloc).astype(np.int64)
    counts = np.bincount(gw, minlength=N_CORES * N_WIN)
    t_win_data = max(1, int(np.ceil(counts.max() / P)))
    t_c = max(2, (t_win_data + N_CHUNK - 1) // N_CHUNK)

    # chunk feasibility per edge: lo = highest chunk with base <= s,
    # hi = lowest chunk with s < base + CHUNK_SPAN (consecutive range)
    bases = np.asarray(CHUNK_BASE, np.int64)
    lo = np.searchsorted(bases, src_s, side="right") - 1
    hi = np.searchsorted(bases + CHUNK_SPAN, src_s, side="right")
    starts = np.concatenate([[0], np.cumsum(counts)])
    tl_val = (tgt_s - (core * N_LOC + wloc * P)).astype(np.float32)

    while True:
        cap = t_c * P
        t_win = N_CHUNK * t_c
        t_tot = N_WIN * t_win
        idx16 = np.zeros((N_CORES, N_WIN * N_CHUNK * cap // 16, 16), np.int16)
        tl_flat = np.full(N_CORES * t_tot * P, -1.0, np.float32)
        ok = True
        for g in range(N_CORES * N_WIN):
            a, b = starts[g], starts[g + 1]
            if b - a > N_CHUNK * cap:
                ok = False
                break
            s_g, hi_g, lo_g, tl_g = src_s[a:b], hi[a:b], lo[a:b], tl_val[a:b]
            taken = np.zeros(b - a, bool)
            c_core, w = divmod(g, N_WIN)
            for c in range(N_CHUNK):
                cand = (~taken) & (hi_g <= c) & (c <= lo_g)
                must = cand & (lo_g == c)
                n_must = int(must.sum())
                if n_must > cap:
                    ok = False
                    break
                sel = must.nonzero()[0]
                flex = (cand & ~must).nonzero()[0][: cap - n_must]
                pick = np.concatenate([sel, flex])
                taken[pick] = True
                n = pick.size
                idx = (s_g[pick] - bases[c]).astype(np.int16)
                # wrapped int16 layout: slot s -> [s % 16, s // 16]
                blk = np.zeros(cap, np.int16)
                blk[:n] = idx
                row0 = (w * N_CHUNK + c) * (cap // 16)
                idx16[c_core, row0 : row0 + cap // 16] = blk.reshape(cap // 16, 16)
                # tl slots for this chunk run (pads stay -1)
                base_slot = g * (t_win * P) + c * cap
                tl_flat[base_slot : base_slot + n] = tl_g[pick]
            if not ok or not taken.all():
                ok = ok and bool(taken.all())
                if not ok:
                    break
        if ok:
            break
        t_c += 1

    tl_dev = tl_flat.reshape(N_CORES, t_tot, P).transpose(0, 2, 1)
    iota = np.tile(np.arange(P, dtype=np.float32), (P, 1))
    in_maps = []
    for c in range(N_CORES):
        # idx16[c]: [n16, 16] with slot s of block g at [g*cap/16 + s%16 ...]
        # -> SBUF layout [128 partitions, cols]: block g occupies columns
        # [g*t_c*8, (g+1)*t_c*8), partition rows 0..15
        n_blocks = N_WIN * N_CHUNK
        cols16 = t_c * 8
        a = idx16[c].reshape(n_blocks, cap // 16, 16)  # [g, col, row]
        # [16, cols] block replicated across all 8 GPSIMD Q7 cores' stripes
        sb = np.tile(a.transpose(2, 0, 1).reshape(16, n_blocks * cols16), (8, 1))
        xT_c = np.zeros((D, N_PAD), np.float32)
        xT_c[:, :N_LOC] = x[c * N_LOC : (c + 1) * N_LOC].T
        consts = np.concatenate(
            [
                sb.view(np.int32),
                tl_dev[c].view(np.int32),
                iota.view(np.int32),
                W.view(np.int32),
                W_self.view(np.int32),
            ],
            axis=1,
        )
        in_maps.append({"x": x, "xT": xT_c, "consts": consts})
    return in_maps, t_win


def run(x, edge_index, W, W_self, trace=False, **trace_kwargs):
    """Returns (output [100000,128] float32, BassKernelResults)."""
    from concourse import bass_utils

    in_maps, t_win = _prep_inputs(x, edge_index, W, W_self)
    nc = _program_cache.get(t_win)
    if nc is None:
        nc = _build_program(t_win)
        _program_cache[t_win] = nc
    res = bass_utils.run_bass_kernel_spmd(
        nc, in_maps, core_ids=list(range(N_CORES)), trace=trace, **trace_kwargs
    )
    out = np.empty((N_NODES, D), np.float32)
    for c in range(N_CORES):
        out[c * N_LOC : (c + 1) * N_LOC] = res.results[c]["outT"].T[:N_LOC]
    return out, res


def kernel(x, edge_index, W, W_self):
    out, _ = run(x, edge_index, W, W_self, trace=False)
    return out
